# revision 1
# baseline (speedup 1.0000x reference)
"""Trainium2 Bass kernel for nn_Attention_30562987278646.

Sharding: 8 cores = 4 batches x 2 head-groups (4 heads each).

Key design points (vs. the f32r baseline):
 - fp16 datapath: matmuls run 1.0 cycles/row at any free size, DVE gets
   2x modes, DMA bytes halve.
 - x loaded via XBAR DMA hardware transpose (fp16-only) straight into
   c-major layout -- no PE transposes, no LN-apply pass.
 - LayerNorm folded post-projection: f^T = r_n*(P - (u/512) x A_n) + bW,
   where P = x @ W', A = colsum(x), r = 512/sqrt(D), D = 512*B - A^2 +
   512^2*eps, B = colsum(x^2).  The rank-2 correction [-u/512; bW/512] x
   [A; sqrtD] rides the projection PSUM accumulation as one K=33 matmul;
   the r_n scale rides the PSUM->SBUF copy as a tensor_mul with a
   broadcast row built by one PE matmul.
 - Scores per head via per-head stacked operands: KSt[h] = [fkn_h; fk_h],
   QSt[h] = [cos_w-scaled fqn_h; (cov_w/64)-scaled fq_h]: one K=128
   matmul gives cos+cov main terms, one K=33 f16 matmul adds the rank-2
   terms (-cov_w*mq x mk and vr x ones).  2 matmuls/score tile vs 4.
 - Paired norm stats: selector columns in `onec` write both heads'
   sq-colsums into one [33,512] PSUM tile (rows 0/32, zero-filled
   between), so Sqrt reads PSUM directly -- no row staging copies.
 - var term: relu(1-cos) == 1-cos since cos<=1; colsum(cos) via a
   [64,1] x [64,N] matmul per head.
 - Stage C and stage D interleave per head-pair so the PE stays fed
   while DVE/Act run the normalization chains of the next pair.
 - out stage: po[64,512] accumulated over 8 m-tiles; W_out applied from
   head-pair-stacked oT2 tiles (K=128, 2 matmuls per n-tile).
 - All DMA issued from the SP sequencer (HWDGE); Pool takes SBUF-only
   squares (GPSIMD has no PSUM port).
"""

import sys
import numpy as np

for _p in ("/opt/trn_rl_repo", "/root/.axon_site/_ro/trn_rl_repo"):
    if _p not in sys.path:
        sys.path.append(_p)

HEADS = 8
DIM_HEAD = 64
LN_EPS = 1e-5
B, N, DIM = 4, 1024, 512
HG = 2                      # head groups (shards along heads)
HPG = HEADS // HG           # heads per group = 4
IG = HPG * DIM_HEAD         # inner dim per group = 256
NT = N // 128               # 8 n-tiles
NC2 = N // 512              # 2 n-chunks
CC = DIM // 128             # 4 c-chunks


def _build_nc(cos_w: float, cov_w: float, var_w: float):
    import concourse.bass as bass
    import concourse.bacc as bacc
    import concourse.tile as tile
    from concourse import mybir

    f32 = mybir.dt.float32
    f32r = mybir.dt.float32r
    f16 = mybir.dt.float16
    AF = mybir.ActivationFunctionType
    AX = mybir.AxisListType
    OP = mybir.AluOpType

    nc = bacc.Bacc(target_bir_lowering=False, debug=False)
    _lp = nc.allow_low_precision(reason="fp16 datapath, 2e-2 tolerance")
    _lp.__enter__()

    xin_d = {
        "xq": nc.declare_dram_parameter("xq", [N, DIM], f16, isOutput=False),
        "xk": nc.declare_dram_parameter("xk", [N, DIM], f16, isOutput=False),
        "xv": nc.declare_dram_parameter("xv", [N, DIM], f16, isOutput=False),
    }
    wf = nc.declare_dram_parameter("wf", [DIM, IG], f16, isOutput=False)
    wo = nc.declare_dram_parameter("wo", [IG, DIM], f16, isOutput=False)
    u2 = nc.declare_dram_parameter("u2", [33, IG], f32, isOutput=False)
    lnk = nc.declare_dram_parameter("lnk", [1, 128], f32, isOutput=False)
    lnq = nc.declare_dram_parameter("lnq", [1, 128], f32, isOutput=False)
    pselk = nc.declare_dram_parameter("pselk", [33, 128], f32, isOutput=False)
    pselq = nc.declare_dram_parameter("pselq", [33, 128], f32, isOutput=False)
    onec = nc.declare_dram_parameter("onec", [128, 67], f16, isOutput=False)
    outp = nc.declare_dram_parameter("out", [N, DIM], f16, isOutput=True)

    with tile.TileContext(nc) as tc, \
         tc.tile_pool(name="persist", bufs=1) as P, \
         tc.tile_pool(name="sq", bufs=6) as SQ, \
         tc.tile_pool(name="stt", bufs=3) as STP, \
         tc.tile_pool(name="osb", bufs=4) as OSB, \
         tc.tile_pool(name="psu", bufs=3, space="PSUM") as PSU, \
         tc.tile_pool(name="pss", bufs=2, space="PSUM") as PSS, \
         tc.tile_pool(name="ppo", bufs=1, space="PSUM") as PPO:

        # ---- constants ----
        wf_sb = [P.tile([128, IG], f16, tag=f"wf{c}", name=f"wf{c}")
                 for c in range(CC)]
        wo_sb = [P.tile([128, DIM], f16, tag=f"wo{j}", name=f"wo{j}")
                 for j in range(2)]
        onec_sb = P.tile([128, 67], f16, tag="onec", name="onec")
        nc.sync.dma_start(out=onec_sb, in_=onec[:, :])
        u2_sb = P.tile([33, IG], f32r, tag="u2", name="u2")
        nc.sync.dma_start(out=u2_sb, in_=u2[:, :].bitcast(f32r))
        lnk_sb = P.tile([1, 128], f32r, tag="lnk", name="lnk")
        nc.sync.dma_start(out=lnk_sb, in_=lnk[:, :].bitcast(f32r))
        lnq_sb = P.tile([1, 128], f32r, tag="lnq", name="lnq")
        nc.sync.dma_start(out=lnq_sb, in_=lnq[:, :].bitcast(f32r))
        pk_sb = P.tile([33, 128], f32r, tag="pselk", name="pselk")
        nc.sync.dma_start(out=pk_sb, in_=pselk[:, :].bitcast(f32r))
        pq_sb = P.tile([33, 128], f32r, tag="pselq", name="pselq")
        nc.sync.dma_start(out=pq_sb, in_=pselq[:, :].bitcast(f32r))

        epsC = P.tile([1, 1], f32, tag="epsC", name="epsC")
        nc.vector.memset(epsC, float(DIM * DIM * LN_EPS))
        epsB = P.tile([33, 1], f32, tag="epsB", name="epsB")
        nc.vector.memset(epsB, 1e-6)
        vw_sb = P.tile([1, 1], f32, tag="vw", name="vw")
        nc.vector.memset(vw_sb, var_w)
        id1 = P.tile([1, 1], f32, tag="id1", name="id1")
        nc.vector.memset(id1, 1.0)

        # ---- persistent activation tiles ----
        xT = {t: [P.tile([128, N], f16, tag=f"xT{t}{c}", name=f"xT{t}{c}")
                  for c in range(CC)] for t in ("xq", "xk", "xv")}
        srow2 = {t: P.tile([33, N], f32r, tag=f"sr{t}", name=f"sr{t}")
                 for t in ("xq", "xk", "xv")}    # row0 = A, row32 = sqrtD
        rrow = {t: P.tile([1, N], f32r if t != "xv" else f32,
                          tag=f"rr{t}", name=f"rr{t}")
                for t in ("xq", "xk", "xv")}      # 1/sqrt(D)
        rb_sb = {t: P.tile([128, N], f16, tag=f"rb{t}", name=f"rb{t}")
                 for t in ("xq", "xk")}
        KSt = [P.tile([128, N], f16, tag=f"KSt{h}", name=f"KSt{h}")
               for h in range(HPG)]               # [fkn_h ; fk_h]
        QSt = [P.tile([128, N], f16, tag=f"QSt{h}", name=f"QSt{h}")
               for h in range(HPG)]               # [fqn_h ; (cov/64) fq_h]
        KS2 = [P.tile([33, N], f16, tag=f"KS2{h}", name=f"KS2{h}")
               for h in range(HPG)]
        QS2 = [P.tile([33, N], f16, tag=f"QS2{h}", name=f"QS2{h}")
               for h in range(HPG)]
        csq = {s: [P.tile([33, N], f32, tag=f"csq{s}{hp}", name=f"csq{s}{hp}")
                   for hp in range(2)] for s in ("k", "q")}
        rws = {s: [P.tile([33, N], f32r, tag=f"rws{s}{hp}",
                          name=f"rws{s}{hp}")
                   for hp in range(2)] for s in ("k", "q")}
        fks = [P.tile([64, 1], f16, tag=f"fks{h}", name=f"fks{h}")
               for h in range(HPG)]
        fkp = [[P.tile([64, 1], f32, tag=f"fkp{h}{x}", name=f"fkp{h}{x}")
                for x in range(NC2)] for h in range(HPG)]
        rvc = [P.tile([128, 1], f32, tag=f"rvc{mt}", name=f"rvc{mt}")
               for mt in range(NT)]
        fv_sb = [P.tile([128, IG], f16, tag=f"fv{mt}", name=f"fv{mt}")
                 for mt in range(NT)]
        oT2 = [P.tile([128, N], f16, tag=f"oT2{g}", name=f"oT2{g}")
               for g in range(2)]
        t1row = P.tile([1, N], f32, tag="t1row", name="t1row")
        d2row = P.tile([1, N], f32, tag="d2row", name="d2row")
        zrow = P.tile([33, N], f32, tag="zrow", name="zrow")

        nc.vector.memset(zrow, 0.0)
        for h in range(HPG):
            nc.vector.memset(KS2[h], 0.0)
            nc.vector.memset(KS2[h][32:33, :], 1.0)
            nc.vector.memset(QS2[h], 0.0)
        for t in ("xq", "xk", "xv"):
            nc.scalar.activation(srow2[t], zrow, AF.Copy)

        # ---- stage A: transposed loads + LN stats ----
        for c in range(CC):
            nc.sync.dma_start(out=wf_sb[c], in_=wf[c * 128:(c + 1) * 128, :])
        for j in range(2):
            nc.sync.dma_start(out=wo_sb[j], in_=wo[j * 128:(j + 1) * 128, :])
        for t in ("xq", "xk", "xv"):
            for c in range(CC):
                nc.sync.dma_start_transpose(
                    out=xT[t][c], in_=xin_d[t][:, c * 128:(c + 1) * 128])

        for t in ("xq", "xk", "xv"):
            sqx = [SQ.tile([128, N], f16, tag="sqx", name=f"sqx{t}{c}")
                   for c in range(CC)]
            for c in range(CC):
                eng = nc.gpsimd if (t != "xv" and c % 2 == 1) else nc.vector
                eng.tensor_mul(sqx[c], xT[t][c], xT[t][c])
            for ncx in range(NC2):
                cs = slice(ncx * 512, (ncx + 1) * 512)
                pA = PSU.tile([128, 512], f32, tag="psu", name=f"pA{t}{ncx}")
                pB = PSU.tile([128, 512], f32, tag="psu", name=f"pB{t}{ncx}")
                for c in range(CC):
                    nc.tensor.matmul(pA[0:1, :], onec_sb[:, 0:1],
                                     xT[t][c][:, cs],
                                     start=(c == 0), stop=(c == CC - 1))
                    nc.tensor.matmul(pB[0:1, :], onec_sb[:, 0:1],
                                     sqx[c][:, cs],
                                     start=(c == 0), stop=(c == CC - 1))
                # A row (f32r, feeds rank-2 matmul + t1)
                nc.scalar.activation(srow2[t][0:1, cs], pA[0:1, :], AF.Copy)
                # t1 = A^2 ; D = 512*B - A^2 ; sqrtD ; rr = 1/sqrtD
                nc.gpsimd.tensor_mul(t1row[:, cs], srow2[t][0:1, cs],
                                     srow2[t][0:1, cs])
                nc.vector.scalar_tensor_tensor(
                    d2row[:, cs], pB[0:1, :], float(DIM), t1row[:, cs],
                    op0=OP.mult, op1=OP.subtract)
                nc.scalar.activation(srow2[t][32:33, cs], d2row[:, cs],
                                     AF.Sqrt, bias=epsC)
                nc.vector.reciprocal(rrow[t][:, cs], srow2[t][32:33, cs])

        # ---- stage B/C/D pipelined per head-pair ----
        # q, k (d-major): pf = P^T - (u/512) x A + (bW/512) x sqrtD,
        # then fT = pf * rb  (rb = c * 512 * rr broadcast)
        for t, side in (("xk", "k"), ("xq", "q")):
            lnsel = lnk_sb if side == "k" else lnq_sb
            for ncx in range(NC2):
                cs = slice(ncx * 512, (ncx + 1) * 512)
                pb = PSU.tile([128, 512], f32, tag="psu")
                nc.tensor.matmul(pb, lnsel, rrow[t][:, cs],
                                 start=True, stop=True)
                nc.scalar.activation(rb_sb[t][:, cs], pb, AF.Copy)

        def emit_projkq(hp):
            for t, side in (("xk", "k"), ("xq", "q")):
                dst = KSt if side == "k" else QSt
                for ncx in range(NC2):
                    cs = slice(ncx * 512, (ncx + 1) * 512)
                    pf = PSU.tile([128, 512], f32, tag="psu")
                    for c in range(CC):
                        nc.tensor.matmul(
                            pf, wf_sb[c][:, hp * 128:(hp + 1) * 128],
                            xT[t][c][:, cs], start=(c == 0), stop=False)
                    nc.tensor.matmul(
                        pf, u2_sb[:, hp * 128:(hp + 1) * 128],
                        srow2[t][:, cs], start=False, stop=True)
                    nc.vector.tensor_mul(dst[2 * hp][64:128, cs],
                                         pf[0:64, :], rb_sb[t][0:64, cs])
                    nc.vector.tensor_mul(dst[2 * hp + 1][64:128, cs],
                                         pf[64:128, :], rb_sb[t][64:128, cs])

        def emit_projv():
            # v (m-major): pf_v = P - A x (u/512) + sqrtD x (bW/512),
            # then fv = pf_v * rv (per-partition col = rr via transpose, x512)
            for mt in range(NT):
                ms = slice(mt * 128, (mt + 1) * 128)
                pc = PSU.tile([128, 512], f32, tag="psu")
                nc.tensor.transpose(pc[:, 0:1], rrow["xv"][:, ms], id1)
                nc.vector.tensor_copy(rvc[mt], pc[:, 0:1])
                pf = PSU.tile([128, 512], f32, tag="psu")
                for c in range(CC):
                    nc.tensor.matmul(pf[:, 0:IG], xT["xv"][c][:, ms],
                                     wf_sb[c], start=(c == 0), stop=False)
                nc.tensor.matmul(pf[:, 0:IG], srow2["xv"][:, ms],
                                 u2_sb, start=False, stop=True)
                nc.vector.tensor_scalar(fv_sb[mt], pf[:, 0:IG], rvc[mt],
                                        float(DIM), op0=OP.mult, op1=OP.mult)

        def emit_C_norm(hp):
            # squares of the pair's four per-head f tiles
            sqs = {}
            for hj in range(2):
                h = 2 * hp + hj
                sk = SQ.tile([64, N], f16, tag="sqh", name=f"sqk{h}")
                nc.vector.tensor_mul(sk, KSt[h][64:128, :],
                                     KSt[h][64:128, :])
                sqs[("k", hj)] = sk
                sq_ = SQ.tile([64, N], f16, tag="sqh", name=f"sqq{h}")
                nc.vector.tensor_mul(sq_, QSt[h][64:128, :],
                                     QSt[h][64:128, :])
                sqs[("q", hj)] = sq_
            # paired sq-colsums -> [33,512] PSUM (rows 0/32) -> Sqrt -> recip
            for side in ("k", "q"):
                for ncx in range(NC2):
                    cs = slice(ncx * 512, (ncx + 1) * 512)
                    p33 = PSU.tile([128, 512], f32, tag="psu")
                    nc.tensor.matmul(p33[0:33, :], onec_sb[0:64, 1:34],
                                     sqs[(side, 0)][:, cs],
                                     start=True, stop=False)
                    nc.tensor.matmul(p33[0:33, :], onec_sb[0:64, 34:67],
                                     sqs[(side, 1)][:, cs],
                                     start=False, stop=True)
                    nc.scalar.activation(csq[side][hp][:, cs], p33[0:33, :],
                                         AF.Sqrt, bias=epsB)
                nc.vector.reciprocal(rws[side][hp], csq[side][hp])
            # normalized rows: fkn = fk * rk, fqn(eff) = fq' * (cos_w/|fq'|)
            for ncx in range(NC2):
                cs = slice(ncx * 512, (ncx + 1) * 512)
                pbk = PSU.tile([128, 512], f32, tag="psu")
                nc.tensor.matmul(pbk, pk_sb, rws["k"][hp][:, cs],
                                 start=True, stop=True)
                pq1 = PSU.tile([128, 512], f32, tag="psu")
                nc.tensor.matmul(pq1[0:64, :], pq_sb[:, 64:128],
                                 rws["q"][hp][:, cs], start=True, stop=True)
                pq2 = PSU.tile([128, 512], f32, tag="psu")
                nc.tensor.matmul(pq2[0:64, :], pq_sb[:, 0:64],
                                 rws["q"][hp][:, cs], start=True, stop=True)
                pba = SQ.tile([128, 512], f16, tag="pbqs",
                              name=f"pba{hp}{ncx}")
                nc.scalar.activation(pba[64:128, :], pq1[0:64, :], AF.Copy)
                pbb = SQ.tile([128, 512], f16, tag="pbqs",
                              name=f"pbb{hp}{ncx}")
                nc.scalar.activation(pbb[64:128, :], pq2[0:64, :], AF.Copy)
                for hj in range(2):
                    h = 2 * hp + hj
                    pp = slice(hj * 64, hj * 64 + 64)
                    nc.vector.scalar_tensor_tensor(
                        KSt[h][0:64, cs], KSt[h][64:128, cs], 1.0,
                        pbk[pp, :], op0=OP.mult, op1=OP.mult,
                        accum_out=fkp[h][ncx])
                    src_pb = pba if hj == 0 else pbb
                    nc.gpsimd.tensor_mul(QSt[h][0:64, cs],
                                         QSt[h][64:128, cs],
                                         src_pb[64:128, :])
            # per-head mean rows (-cov_w*mq), mk columns, var rows
            for hj in range(2):
                h = 2 * hp + hj
                hb = h * N
                for ncx in range(NC2):
                    cs = slice(ncx * 512, (ncx + 1) * 512)
                    pm = PSU.tile([128, 512], f32, tag="psu")
                    nc.tensor.matmul(pm[0:1, :], onec_sb[64:128, 0:1],
                                     KSt[h][64:128, cs],
                                     start=True, stop=True)
                    pm2 = PSU.tile([128, 512], f32, tag="psu")
                    nc.tensor.matmul(pm2[0:1, :], onec_sb[64:128, 0:1],
                                     QSt[h][64:128, cs],
                                     start=True, stop=True)
                    nc.scalar.activation(KS2[h][0:1, cs], pm[0:1, :],
                                         AF.Identity, scale=1.0 / DIM_HEAD)
                    if ncx == 0:
                        nc.scalar.activation(QS2[h][0:1, cs], pm2[0:1, :],
                                             AF.Identity, scale=-1.0)
                    else:
                        nc.vector.tensor_scalar_mul(QS2[h][0:1, cs],
                                                    pm2[0:1, :], -1.0)
                nc.vector.tensor_add(fks[h], fkp[h][0], fkp[h][1])
                for ncx in range(NC2):
                    cs = slice(ncx * 512, (ncx + 1) * 512)
                    pv = PSU.tile([128, 512], f32, tag="psu")
                    nc.tensor.matmul(pv[0:1, :], fks[h], QSt[h][0:64, cs],
                                     start=True, stop=True)
                    if ncx == 0:
                        nc.scalar.activation(QS2[h][32:33, cs], pv[0:1, :],
                                             AF.Identity, bias=vw_sb,
                                             scale=-(var_w / (N * cos_w)))
                    else:
                        nc.vector.tensor_scalar(QS2[h][32:33, cs],
                                                pv[0:1, :],
                                                -(var_w / (N * cos_w)),
                                                var_w, op0=OP.mult,
                                                op1=OP.add)

        def emit_E(nts):
            for nt in nts:
                ns = slice(nt * 128, (nt + 1) * 128)
                pf = PSU.tile([128, 512], f32, tag="psu")
                nc.tensor.matmul(pf, oT2[0][:, ns], wo_sb[0],
                                 start=True, stop=False)
                nc.tensor.matmul(pf, oT2[1][:, ns], wo_sb[1],
                                 start=False, stop=True)
                ob = OSB.tile([128, DIM], f16, tag="ob")
                nc.vector.tensor_copy(ob, pf)
                nc.sync.dma_start(out=outp[ns, :], in_=ob)

        _di = [0]

        def emit_D(heads):
            for ncx in range(NC2):
                for h in heads:
                    hp, hj = h // 2, h % 2
                    hb = h * N
                    cs = slice(ncx * 512, (ncx + 1) * 512)
                    q2 = slice(hb + ncx * 512, hb + ncx * 512 + 512)
                    po = PPO.tile([64, 512], f32, tag="po")
                    for mtp in range(NT // 2):
                        pss = PSS.tile([128, 1024], f32, tag="pss")
                        for j in range(2):
                            mt = 2 * mtp + j
                            ms = slice(mt * 128, (mt + 1) * 128)
                            half = pss[:, j * 512:(j + 1) * 512]
                            nc.tensor.matmul(half,
                                             KSt[h][:, ms], QSt[h][:, cs],
                                             start=True, stop=False)
                            nc.tensor.matmul(
                                half, KS2[h][:, ms], QS2[h][:, cs],
                                start=False, stop=True)
                        st = STP.tile([128, 1024], f16, tag="st")
                        if _di[0] % 8 < 5:
                            nc.scalar.activation(st, pss, AF.Copy)
                        else:
                            nc.vector.tensor_copy(st, pss)
                        _di[0] += 1
                        for j in range(2):
                            mt = 2 * mtp + j
                            nc.tensor.matmul(
                                po, fv_sb[mt][:, h * 64:(h + 1) * 64],
                                st[:, j * 512:(j + 1) * 512],
                                start=(mtp == 0 and j == 0),
                                stop=(mtp == NT // 2 - 1 and j == 1))
                    nc.scalar.activation(
                        oT2[hp][hj * 64:hj * 64 + 64, cs], po, AF.Copy)
                if heads[0] == 2:
                    emit_E(range(ncx * 4, ncx * 4 + 4))


        emit_projkq(0)
        emit_projkq(1)
        emit_projv()
        emit_C_norm(0)
        emit_D((0, 1))
        emit_C_norm(1)
        emit_D((2, 3))

    _lp.__exit__(None, None, None)
    nc.compile()
    return nc


def _prep(q, k, v, ln_g, ln_b, W_in, W_out, b_out, cov_w_raw, var_w_raw):
    q = np.asarray(q, np.float32)
    k = np.asarray(k, np.float32)
    v = np.asarray(v, np.float32)
    ln_g = np.asarray(ln_g, np.float32)
    ln_b = np.asarray(ln_b, np.float32)
    W_in = np.asarray(W_in, np.float32)
    W_out = np.asarray(W_out, np.float32)

    cov_w = float(1.0 / (1.0 + np.exp(-np.float64(cov_w_raw))))
    var_w = float(1.0 / (1.0 + np.exp(-np.float64(var_w_raw))))
    cos_w = 1.0 - cov_w - var_w

    nc = _build_nc(cos_w, cov_w, var_w)

    W_f = (ln_g[:, None] * W_in).astype(np.float32)      # [512, 512]
    bW = (ln_b @ W_in).astype(np.float32)                # [512]
    u = W_f.sum(axis=0)                                  # [512]

    lnka = np.full((1, 128), float(DIM), np.float32)
    lnqa = np.full((1, 128), float(DIM) * cov_w / DIM_HEAD, np.float32)
    pka = np.zeros((33, 128), np.float32)
    pka[0, :64] = 1.0
    pka[32, 64:] = 1.0
    pqa = np.zeros((33, 128), np.float32)
    pqa[0, 64:] = cos_w     # head-even rq -> out partitions 0:64 of pq1
    pqa[32, :64] = cos_w    # head-odd rq -> out partitions 0:64 of pq2
    onec = np.zeros((128, 67), np.float16)
    onec[:, 0] = 1.0
    onec[:64, 1] = 1.0
    onec[:64, 66] = 1.0

    qh = [np.ascontiguousarray(q[b]).astype(np.float16) for b in range(B)]
    kh = [np.ascontiguousarray(k[b]).astype(np.float16) for b in range(B)]
    vh = [np.ascontiguousarray(v[b]).astype(np.float16) for b in range(B)]

    in_maps = []
    for core in range(8):
        b, g = core // HG, core % HG
        gs = slice(g * IG, (g + 1) * IG)
        u2a = np.zeros((33, IG), np.float32)
        u2a[0] = -u[gs] / DIM
        u2a[32] = bW[gs] / DIM
        in_maps.append({
            "xq": qh[b], "xk": kh[b], "xv": vh[b],
            "wf": np.ascontiguousarray(W_f[:, gs]).astype(np.float16),
            "wo": np.ascontiguousarray(W_out[gs, :]).astype(np.float16),
            "u2": u2a, "lnk": lnka, "lnq": lnqa,
            "pselk": pka, "pselq": pqa, "onec": onec,
        })
    return nc, in_maps


def kernel(q, k, v, ln_g, ln_b, W_in, W_out, b_out, cov_w_raw, var_w_raw):
    from concourse.bass_utils import run_bass_kernel_spmd

    b_out = np.asarray(b_out, np.float32)
    nc, in_maps = _prep(q, k, v, ln_g, ln_b, W_in, W_out, b_out,
                        cov_w_raw, var_w_raw)
    res = run_bass_kernel_spmd(nc, in_maps, list(range(8)))
    parts = [res.results[c]["out"].astype(np.float32) for c in range(8)]
    out = np.stack([parts[2 * b] + parts[2 * b + 1] + b_out
                    for b in range(B)])
    return out.astype(np.float32)

